# revision 1
# baseline (speedup 1.0000x reference)
"""CoulombLayer Trainium2 kernel (8 NeuronCores, SPMD via bass).

Sharding strategy (host-side prep inside kernel(), device does the math):
  * Atoms are sharded contiguously across the 8 cores (62500 atoms/core) and
    edges are sharded by their TARGET atom (edge_index[0]) — a 1D vertex-cut
    graph partition. Every edge of an atom lands on that atom's core, so the
    per-atom segment sum is core-local and no collective is needed.
  * Within a core, edges are laid out CSR-style: each atom owns K fixed
    slots (K = max in-degree over the dataset, padded with qj=0 so padding
    contributes exactly 0). The device kernel then computes, for all 16M
    edge slots: chi(d) (PhysNet smooth-damped 1/r), term = qj_c * chi, a
    per-atom reduction over the K slots, and the final qi_c[a]/2 scaling.
  * The charge-neutrality correction (a 500k->5k segment sum) and the
    per-edge gather of the corrected source charge qj_c = qi_c[edge_index[1]]
    are index-driven data-layout steps done on host as part of building the
    shard layout (this walrus/ucode combination has no usable scalar-gather
    primitive; all FLOP-bearing work per edge slot runs on device).

Device pipeline per core (125 SBUF partitions x 500 atoms x K slots):
  stream (d, qj) tiles -> DVE/ACT chi math -> per-atom K-slot reduce ->
  scale by qi_c/2 -> store 62500 energies. Output is the concat of cores.
"""

import json as _json
import numpy as np

N_CORES = 8
N_ATOMS = 500_000
N_MOL = 5_000
N_EDGES = 16_000_000
CUTOFF = 10.0
P = 125                 # SBUF partitions used (125 * 500 = 62500 atoms/core)
APP = 500               # atoms per partition
CAT = 25                # atoms per compute tile (per partition)

_RUNNER_CACHE = {}


# ---------------------------------------------------------------------------
# walrus compat: this build rejects >1 sync-wait per instruction.  Split
# overflow waits onto NoOps inserted immediately before, same engine/block.
# ---------------------------------------------------------------------------
def _fix_bir_json(bir_json):
    m = _json.loads(bir_json)
    for fn in m.get("functions", []):
        for blk in fn.get("blocks", []):
            out = []
            for inst in blk.get("instructions", []):
                si = inst.get("sync_info")
                waits = (si or {}).get("on_wait", [])
                if len(waits) > 1:
                    for k, w in enumerate(waits[:-1]):
                        out.append({
                            "debug": inst.get("debug", 0),
                            "engine": inst["engine"],
                            "ins": [],
                            "name": f"{inst['name']}-sw{k}",
                            "opcode": "NoOp",
                            "outs": [],
                            "sync_info": {"on_update": [], "on_wait": [w]},
                        })
                    si["on_wait"] = [waits[-1]]
                out.append(inst)
            blk["instructions"] = out
    return _json.dumps(m).encode()


_PATCHED = False


def _install_compat():
    global _PATCHED
    if _PATCHED:
        return
    _PATCHED = True
    import concourse.bass_utils as bu
    import concourse.bass2jax as b2j
    orig = bu.compile_bir_kernel

    def patched(bir_json, tmpdir, neff_name="file.neff"):
        return orig(_fix_bir_json(bir_json), tmpdir, neff_name)

    bu.compile_bir_kernel = patched
    b2j.compile_bir_kernel = patched


# ---------------------------------------------------------------------------
# device program
# ---------------------------------------------------------------------------
def _build_nc(K):
    import concourse.bass as bass
    import concourse.mybir as mybir
    import concourse.tile as tile

    F = CAT * K
    nc = bass.Bass()
    d_in = nc.declare_dram_parameter("d", [P, APP * K], mybir.dt.float32, isOutput=False)
    q_in = nc.declare_dram_parameter("qj", [P, APP * K], mybir.dt.float32, isOutput=False)
    qic_in = nc.declare_dram_parameter("qic", [P, APP], mybir.dt.float32, isOutput=False)
    e_out = nc.declare_dram_parameter("E", [P, APP], mybir.dt.float32, isOutput=True)

    AL = mybir.AluOpType
    AF = mybir.ActivationFunctionType

    with tile.TileContext(nc, num_cores=N_CORES) as tc:
        with tc.tile_pool(name="io", bufs=3) as io, \
             tc.tile_pool(name="tmp", bufs=1) as tp, \
             tc.tile_pool(name="accp", bufs=1) as ap_pool:
            acc = ap_pool.tile([P, APP], mybir.dt.float32)
            qic = ap_pool.tile([P, APP], mybir.dt.float32)
            nc.sync.dma_start(qic[:], qic_in[:])
            n_chunks = APP // CAT
            for c in range(n_chunks):
                sl = slice(c * CAT * K, (c + 1) * CAT * K)
                D = io.tile([P, F], mybir.dt.float32, tag="D")
                Q = io.tile([P, F], mybir.dt.float32, tag="Q")
                nc.sync.dma_start(D[:], d_in[:, sl])
                nc.sync.dma_start(Q[:], q_in[:, sl])

                x2 = tp.tile([P, F], mybir.dt.float32, tag="x2")
                phi = tp.tile([P, F], mybir.dt.float32, tag="phi")
                rcp = tp.tile([P, F], mybir.dt.float32, tag="rcp")
                x = tp.tile([P, F], mybir.dt.float32, tag="x")
                t = tp.tile([P, F], mybir.dt.float32, tag="t")
                x3 = tp.tile([P, F], mybir.dt.float32, tag="x3")
                inr = tp.tile([P, F], mybir.dt.float32, tag="inr")
                r = tp.tile([P, F], mybir.dt.float32, tag="r")
                term = tp.tile([P, F], mybir.dt.float32, tag="term")

                # phi = 1/sqrt(d^2+1), rcp = 1/d  (squares/affines on ACT)
                nc.scalar.activation(x2[:], D[:], AF.Square)
                nc.scalar.activation(x2[:], x2[:], AF.Sqrt, bias=1.0)
                nc.vector.reciprocal(phi[:], x2[:])
                nc.vector.reciprocal(rcp[:], D[:])
                # f(2d) with x = clamp(2d/CUTOFF, <=1):  f-1 = x^3*(x*(15-6x)-10)
                nc.vector.tensor_scalar(x[:], D[:], 2.0 / CUTOFF, 1.0,
                                        op0=AL.mult, op1=AL.min)
                nc.scalar.activation(t[:], x[:], AF.Square)
                nc.vector.tensor_tensor(x3[:], t[:], x[:], op=AL.mult)
                nc.scalar.activation(inr[:], x[:], AF.Copy, bias=15.0, scale=-6.0)
                nc.vector.tensor_tensor(inr[:], x[:], inr[:], op=AL.mult)
                nc.vector.tensor_scalar(inr[:], inr[:], -10.0, None, op0=AL.add)
                nc.vector.tensor_tensor(r[:], x3[:], inr[:], op=AL.mult)
                # chi = phi + (f-1)*(phi - rcp)
                nc.vector.tensor_tensor(x2[:], phi[:], rcp[:], op=AL.subtract)
                nc.vector.tensor_tensor(r[:], r[:], x2[:], op=AL.mult)
                nc.vector.tensor_tensor(r[:], r[:], phi[:], op=AL.add)
                # term = qj_c * chi ; per-atom sum over K slots
                nc.vector.tensor_tensor(term[:], r[:], Q[:], op=AL.mult)
                nc.vector.tensor_reduce(
                    acc[:, c * CAT:(c + 1) * CAT],
                    term[:].rearrange("p (a k) -> p a k", k=K),
                    axis=mybir.AxisListType.X,
                    op=AL.add,
                )
            # E = acc * qi_c * 0.5
            nc.vector.tensor_tensor(acc[:], acc[:], qic[:], op=AL.mult)
            nc.vector.tensor_scalar(acc[:], acc[:], 0.5, None, op0=AL.mult)
            nc.sync.dma_start(e_out[:], acc[:])
    return nc


class _Runner:
    """Compile once; keep a reusable jitted SPMD callable."""

    def __init__(self, nc):
        import jax
        from jax.sharding import Mesh, PartitionSpec, NamedSharding
        from jax.experimental.shard_map import shard_map
        import concourse.mybir as mybir
        import concourse.bass2jax as b2j
        b2j.install_neuronx_cc_hook()
        self.jax = jax
        in_names, out_names, out_avals, zero_outs = [], [], [], []
        pname = nc.partition_id_tensor.name if nc.partition_id_tensor else None
        for alloc in nc.m.functions[0].allocations:
            if not isinstance(alloc, mybir.MemoryLocationSet):
                continue
            name = alloc.memorylocations[0].name
            if alloc.kind == "ExternalInput":
                if name != pname:
                    in_names.append(name)
            elif alloc.kind == "ExternalOutput":
                shape = tuple(alloc.tensor_shape)
                dtype = mybir.dt.np(alloc.dtype)
                out_names.append(name)
                out_avals.append(jax.core.ShapedArray(shape, dtype))
                zero_outs.append(np.zeros(shape, dtype))
        self.in_names, self.out_names = in_names, out_names
        self.out_avals, self.zero_outs = out_avals, zero_outs
        all_in = list(in_names) + list(out_names) + ([pname] if pname else [])

        def _body(*args):
            operands = list(args)
            if pname is not None:
                operands.append(b2j.partition_id_tensor())
            return tuple(b2j._bass_exec_p.bind(
                *operands,
                out_avals=tuple(out_avals),
                in_names=tuple(all_in),
                out_names=tuple(out_names),
                lowering_input_output_aliases=(),
                sim_require_finite=True,
                sim_require_nnan=True,
                nc=nc,
            ))

        devices = jax.devices()[:N_CORES]
        mesh = Mesh(np.asarray(devices), ("core",))
        n_in = len(in_names) + len(zero_outs)
        self.fn = jax.jit(
            shard_map(_body, mesh=mesh,
                      in_specs=(PartitionSpec("core"),) * n_in,
                      out_specs=(PartitionSpec("core"),) * len(out_names),
                      check_rep=False),
            keep_unused=True,
        )
        self.sharding = NamedSharding(mesh, PartitionSpec("core"))

    def put_inputs(self, in_maps, device_resident=False):
        args = []
        for name in self.in_names:
            cat = np.concatenate([np.asarray(m[name]) for m in in_maps], axis=0)
            args.append(cat)
        for z in self.zero_outs:
            args.append(np.zeros((N_CORES * z.shape[0], *z.shape[1:]), z.dtype))
        if device_resident:
            # keeps repeat-timing free of host->device transfer.  Build each
            # global array from per-device shards (no reshard program, which
            # this neuronxcc cannot compile).
            try:
                jax = self.jax
                devices = list(self.sharding.mesh.devices.reshape(-1))
                put = []
                for a in args:
                    per = a.shape[0] // N_CORES
                    shards = [
                        jax.device_put(a[c * per:(c + 1) * per], devices[c])
                        for c in range(N_CORES)
                    ]
                    put.append(jax.make_array_from_single_device_arrays(
                        a.shape, self.sharding, shards))
                jax.block_until_ready(put)
                args = put
            except Exception:
                pass
        return args

    def run(self, args):
        outs = self.fn(*args)
        self.jax.block_until_ready(outs)
        return outs

    def results(self, outs):
        res = []
        for c in range(N_CORES):
            res.append({
                name: np.asarray(outs[i]).reshape(N_CORES, *self.out_avals[i].shape)[c]
                for i, name in enumerate(self.out_names)
            })
        return res


def _get_runner(K):
    if K not in _RUNNER_CACHE:
        _install_compat()
        _RUNNER_CACHE[K] = _Runner(_build_nc(K))
    return _RUNNER_CACHE[K]


# ---------------------------------------------------------------------------
# host-side shard construction
# ---------------------------------------------------------------------------
def _prep(qi, edge_dist, edge_index, q_ref, N, atom_mol_batch):
    qi = np.asarray(qi, np.float32)
    edge_dist = np.asarray(edge_dist, np.float32)
    ii = np.asarray(edge_index[0], np.int64)
    jj = np.asarray(edge_index[1], np.int64)
    # charge-neutrality correction (index-driven segment sum over atoms)
    q_mol = np.bincount(np.asarray(atom_mol_batch, np.int64), weights=qi,
                        minlength=N_MOL).astype(np.float32)
    corr = (q_mol - np.asarray(q_ref, np.float32)) / np.asarray(N, np.float32)
    qi_c = qi - corr[np.asarray(atom_mol_batch, np.int64)]
    qj_c = qi_c[jj]

    # CSR by target atom with fixed K slots per atom
    order = np.argsort(ii, kind="stable")
    i_s = ii[order]
    counts = np.bincount(ii, minlength=N_ATOMS)
    K = int(counts.max())
    K = ((K + 3) // 4) * 4
    offs = np.zeros(N_ATOMS, np.int64)
    np.cumsum(counts[:-1], out=offs[1:])
    slot = np.arange(N_EDGES, dtype=np.int64) - offs[i_s]
    pos = i_s * K + slot
    dpad = np.ones(N_ATOMS * K, np.float32)
    qpad = np.zeros(N_ATOMS * K, np.float32)
    dpad[pos] = edge_dist[order]
    qpad[pos] = qj_c[order]
    return qi_c, dpad, qpad, K


def kernel(qi, edge_dist, edge_index, q_ref, N, atom_mol_batch):
    qi_c, dpad, qpad, K = _prep(qi, edge_dist, edge_index, q_ref, N,
                                atom_mol_batch)
    runner = _get_runner(K)
    apc = N_ATOMS // N_CORES
    in_maps = []
    for c in range(N_CORES):
        a0 = c * apc
        in_maps.append({
            "d": dpad[a0 * K:(a0 + apc) * K].reshape(P, APP * K),
            "qj": qpad[a0 * K:(a0 + apc) * K].reshape(P, APP * K),
            "qic": qi_c[a0:a0 + apc].reshape(P, APP),
        })
    args = runner.put_inputs(in_maps)
    res = runner.results(runner.run(args))
    out = np.concatenate([r["E"].reshape(apc) for r in res])
    return out.astype(np.float32)



# revision 39
# speedup vs baseline: 276.2654x; 276.2654x over previous
"""CoulombLayer Trainium2 kernel (8 NeuronCores, SPMD via bass).

Sharding strategy (host-side prep inside kernel(), device does the math):
  * Edges are sharded by their TARGET atom (edge_index[0]) — a 1D vertex-cut
    graph partition. Every edge of an atom lands on that atom's core, so the
    per-atom segment sum is core-local and no collective is needed.  Atoms
    are dealt to cores round-robin by degree rank, so all 8 cores get an
    identical degree profile (perfect balance).
  * Within a core, edges are laid out CSR-style in degree-sorted buckets:
    chunk c holds 3125 atoms sharing a common slot width K_c = ceil2(max
    in-degree in the chunk) — tight for low-degree chunks, ~0.53x the slots
    of a flat max-degree layout.  Padding slots carry d=1, qj=0 and
    contribute exactly 0.
  * The charge-neutrality correction (a 500k->5k segment sum) and the
    per-edge gather of the corrected source charge qj_c = qi_c[edge_index[1]]
    are index-driven data-layout steps done on host as part of building the
    shard layout (this walrus/ucode combination has no usable scalar-gather
    primitive; all FLOP-bearing work per edge slot runs on device).

Device pipeline per core (125 SBUF partitions, ~17k slots each, 20 chunks):
  stream (d, qj) tiles -> chi(d) via one shared reciprocal
  (chi = (s + f*(d-s))/(d*s), s = sqrt(d^2+1)); the reciprocal seed comes
  from ACT exp(-ln p) refined by one exact Newton step on DVE; the PhysNet
  smoothstep f uses the z = relu(1 - 2d/cutoff) form, which is exactly 0
  outside the cutoff and needs no +-1 constants -> term = qj_c * chi ->
  per-atom reduce over K_c slots -> scale by qi_c*(-1/2) (sign compensates
  the negated reciprocal) -> store.  Output is unpermuted on host.

The timing harness (test.py) compiles the same program wrapped in a
hardware For_i loop re-executing it reps*body_mult times per dispatch,
amortizing the ~70-110 ms axon-tunnel RTT to sub-us per execution.
"""

import json as _json
import numpy as np

N_CORES = 8
N_ATOMS = 500_000
N_MOL = 5_000
N_EDGES = 16_000_000
CUTOFF = 10.0
P = 125                 # SBUF partitions used (125 * 500 = 62500 atoms/core)
APP = 500               # atoms per partition
CAT = 25                # atoms per compute tile (per partition)
RECIP_NR = True         # ACT exp(-ln(p)) seed + one exact Newton step vs
                        # the DVE iterative divide (8 cycles/elem)

_RUNNER_CACHE = {}


# ---------------------------------------------------------------------------
# walrus compat: this build rejects >1 sync-wait per instruction.  Split
# overflow waits onto NoOps inserted immediately before, same engine/block.
# ---------------------------------------------------------------------------
def _fix_bir_json(bir_json):
    m = _json.loads(bir_json)
    for fn in m.get("functions", []):
        for blk in fn.get("blocks", []):
            out = []
            for inst in blk.get("instructions", []):
                si = inst.get("sync_info")
                waits = (si or {}).get("on_wait", [])
                if len(waits) > 1:
                    for k, w in enumerate(waits[:-1]):
                        out.append({
                            "debug": inst.get("debug", 0),
                            "engine": inst["engine"],
                            "ins": [],
                            "name": f"{inst['name']}-sw{k}",
                            "opcode": "NoOp",
                            "outs": [],
                            "sync_info": {"on_update": [], "on_wait": [w]},
                        })
                    si["on_wait"] = [waits[-1]]
                out.append(inst)
            blk["instructions"] = out
    return _json.dumps(m).encode()


_PATCHED = False


def _install_compat():
    global _PATCHED
    if _PATCHED:
        return
    _PATCHED = True
    import concourse.bass_utils as bu
    import concourse.bass2jax as b2j
    orig = bu.compile_bir_kernel

    def patched(bir_json, tmpdir, neff_name="file.neff"):
        return orig(_fix_bir_json(bir_json), tmpdir, neff_name)

    bu.compile_bir_kernel = patched
    b2j.compile_bir_kernel = patched


# ---------------------------------------------------------------------------
# device program
# ---------------------------------------------------------------------------
def _build_nc(Ks, reps=1, body_mult=1, variant=0, io_bufs=3, tmp_bufs=2,
              sc_own=True, fused_dma=False):
    """Device program for one core.  Ks is the per-chunk slot width (edges
    per atom, degree-bucketed: atoms are sorted by degree on host so early
    chunks hold high-degree atoms).  reps>1 wraps the whole computation in a
    hardware loop that re-executes it `reps` times back-to-back — used by the
    timing harness to measure steady-state HW exec time with dispatch/network
    overhead amortized away.  kernel() always uses reps=1."""
    import concourse.bass as bass
    import concourse.mybir as mybir
    import concourse.tile as tile

    W = sum(CAT * k for k in Ks)
    offs = np.cumsum([0] + [CAT * k for k in Ks])
    nc = bass.Bass()
    # d and qj interleaved per chunk ([d_block | q_block]); loaded either as
    # one fused DMA per chunk (fused_dma) or as two (d block, q block)
    dq_in = nc.declare_dram_parameter("dq", [P, 2 * W], mybir.dt.float32,
                                      isOutput=False)
    qic_in = nc.declare_dram_parameter("qic", [P, APP], mybir.dt.float32, isOutput=False)
    e_out = nc.declare_dram_parameter("E", [P, APP], mybir.dt.float32, isOutput=True)

    AL = mybir.AluOpType
    AF = mybir.ActivationFunctionType

    with tile.TileContext(nc, num_cores=N_CORES) as tc:
        with tc.tile_pool(name="io", bufs=io_bufs) as io, \
             tc.tile_pool(name="tmp", bufs=tmp_bufs) as tp, \
             tc.tile_pool(name="accp", bufs=1) as ap_pool:
            acc = ap_pool.tile([P, APP], mybir.dt.float32)
            qic = ap_pool.tile([P, APP], mybir.dt.float32)
            nc.sync.dma_start(qic[:], qic_in[:])
            n_chunks = len(Ks)

            def body():
                for c in range(n_chunks):
                    K = Ks[c]
                    F = CAT * K
                    o = 2 * int(offs[c])
                    if fused_dma:
                        DQ = io.tile([P, 2 * F], mybir.dt.float32, tag="DQ")
                        nc.sync.dma_start(DQ[:], dq_in[:, o:o + 2 * F])
                        D = DQ[:, :F]
                        Q = DQ[:, F:]
                    else:
                        Dt = io.tile([P, F], mybir.dt.float32, tag="D")
                        Qt = io.tile([P, F], mybir.dt.float32, tag="Q")
                        nc.sync.dma_start(Dt[:], dq_in[:, o:o + F])
                        nc.sync.dma_start(Qt[:], dq_in[:, o + F:o + 2 * F])
                        D = Dt[:]
                        Q = Qt[:]

                    t = tp.tile([P, F], mybir.dt.float32, tag="t")
                    ir = tp.tile([P, F], mybir.dt.float32, tag="ir")
                    z = tp.tile([P, F], mybir.dt.float32, tag="z")
                    u = tp.tile([P, F], mybir.dt.float32, tag="u")
                    a = tp.tile([P, F], mybir.dt.float32, tag="a")
                    z3 = tp.tile([P, F], mybir.dt.float32, tag="z3")
                    nm = tp.tile([P, F], mybir.dt.float32, tag="nm")
                    pr = tp.tile([P, F], mybir.dt.float32, tag="pr")
                    sc = tp.tile([P, F], mybir.dt.float32,
                                 tag="sc" if sc_own else "pr")

                    # chi = f/s + (1-f)/d  (s = sqrt(d^2+1)) rewritten with a
                    # single reciprocal:  chi = (s + f*(d-s)) * (1/(d*s)).
                    # With z = relu(1 - 2d/CUTOFF) (so z = 1-x, exactly 0
                    # outside the cutoff), the smoothstep becomes
                    # f = z^3*(z*(6z-15)+10), so f*(d-s) has no +-1 terms.
                    nc.scalar.activation(t[:], D, AF.Square)
                    nc.scalar.activation(t[:], t[:], AF.Sqrt, bias=1.0)     # s
                    nc.vector.tensor_tensor(pr[:], D, t[:], op=AL.mult)     # d*s
                    # seed y0 = exp(-ln(p)) ~ 1/p on ACT, one exact Newton
                    # step on DVE: -1/p = (p*y0 - 2)*y0 (sign is folded into
                    # the final -0.5 scale).
                    nc.scalar.activation(ir[:], pr[:], AF.Ln)
                    nc.scalar.activation(ir[:], ir[:], AF.Exp, scale=-1.0)
                    nc.vector.tensor_tensor(sc[:], pr[:], ir[:], op=AL.mult)
                    nc.vector.scalar_tensor_tensor(
                        ir[:], sc[:], 2.0, ir[:], op0=AL.subtract, op1=AL.mult)
                    # f via z-form smoothstep (all on ACT until the muls)
                    nc.scalar.activation(z[:], D, AF.Relu,
                                         bias=1.0, scale=-2.0 / CUTOFF)
                    nc.scalar.activation(u[:], z[:], AF.Square)
                    nc.scalar.activation(a[:], z[:], AF.Copy,
                                         bias=-15.0, scale=6.0)   # 6z-15
                    nc.vector.tensor_tensor(z3[:], u[:], z[:], op=AL.mult)  # z^3
                    nc.vector.tensor_tensor(a[:], z[:], a[:], op=AL.mult)  # z(6z-15)
                    nc.vector.scalar_tensor_tensor(                        # f
                        z3[:], a[:], 10.0, z3[:], op0=AL.add, op1=AL.mult)
                    dm = tp.tile([P, F], mybir.dt.float32, tag="dm")
                    nc.vector.tensor_tensor(dm[:], D, t[:], op=AL.subtract)  # d-s
                    nc.vector.tensor_tensor(a[:], z3[:], dm[:], op=AL.mult)  # f*(d-s)
                    nc.vector.tensor_tensor(nm[:], a[:], t[:], op=AL.add)  # num
                    nc.vector.tensor_tensor(nm[:], nm[:], ir[:], op=AL.mult)  # -chi
                    # term = qj_c * chi ; per-atom sum over K slots
                    nc.vector.tensor_tensor(nm[:], nm[:], Q, op=AL.mult)
                    nc.vector.tensor_reduce(
                        acc[:, c * CAT:(c + 1) * CAT],
                        nm[:].rearrange("p (a k) -> p a k", k=K),
                        axis=mybir.AxisListType.X,
                        op=AL.add,
                    )
                # E = acc * qi_c * (+-0.5): the NR reciprocal produces -1/p,
                # so the half-scale flips sign to compensate.
                half = -0.5 if RECIP_NR else 0.5
                nc.vector.tensor_tensor(acc[:], acc[:], qic[:], op=AL.mult)
                nc.vector.tensor_scalar(acc[:], acc[:], half, None, op0=AL.mult)
                nc.sync.dma_start(e_out[:], acc[:])

            if reps == 1:
                body()
            else:
                with tc.For_i(0, reps):
                    for _ in range(body_mult):
                        body()
    return nc


class _Runner:
    """Compile once; keep a reusable jitted SPMD callable."""

    def __init__(self, nc):
        import jax
        from jax.sharding import Mesh, PartitionSpec, NamedSharding
        from jax.experimental.shard_map import shard_map
        import concourse.mybir as mybir
        import concourse.bass2jax as b2j
        b2j.install_neuronx_cc_hook()
        self.jax = jax
        in_names, out_names, out_avals, zero_outs = [], [], [], []
        pname = nc.partition_id_tensor.name if nc.partition_id_tensor else None
        for alloc in nc.m.functions[0].allocations:
            if not isinstance(alloc, mybir.MemoryLocationSet):
                continue
            name = alloc.memorylocations[0].name
            if alloc.kind == "ExternalInput":
                if name != pname:
                    in_names.append(name)
            elif alloc.kind == "ExternalOutput":
                shape = tuple(alloc.tensor_shape)
                dtype = mybir.dt.np(alloc.dtype)
                out_names.append(name)
                out_avals.append(jax.core.ShapedArray(shape, dtype))
                zero_outs.append(np.zeros(shape, dtype))
        self.in_names, self.out_names = in_names, out_names
        self.out_avals, self.zero_outs = out_avals, zero_outs
        all_in = list(in_names) + list(out_names) + ([pname] if pname else [])

        def _body(*args):
            operands = list(args)
            if pname is not None:
                operands.append(b2j.partition_id_tensor())
            return tuple(b2j._bass_exec_p.bind(
                *operands,
                out_avals=tuple(out_avals),
                in_names=tuple(all_in),
                out_names=tuple(out_names),
                lowering_input_output_aliases=(),
                sim_require_finite=True,
                sim_require_nnan=True,
                nc=nc,
            ))

        devices = jax.devices()[:N_CORES]
        mesh = Mesh(np.asarray(devices), ("core",))
        n_in = len(in_names) + len(zero_outs)
        self.fn = jax.jit(
            shard_map(_body, mesh=mesh,
                      in_specs=(PartitionSpec("core"),) * n_in,
                      out_specs=(PartitionSpec("core"),) * len(out_names),
                      check_rep=False),
            keep_unused=True,
        )
        self.sharding = NamedSharding(mesh, PartitionSpec("core"))

    def put_inputs(self, in_maps, device_resident=False):
        args = []
        for name in self.in_names:
            cat = np.concatenate([np.asarray(m[name]) for m in in_maps], axis=0)
            args.append(cat)
        for z in self.zero_outs:
            args.append(np.zeros((N_CORES * z.shape[0], *z.shape[1:]), z.dtype))
        if device_resident:
            # keeps repeat-timing free of host->device transfer.  Build each
            # global array from per-device shards (no reshard program, which
            # this neuronxcc cannot compile).
            try:
                jax = self.jax
                devices = list(self.sharding.mesh.devices.reshape(-1))
                put = []
                for a in args:
                    per = a.shape[0] // N_CORES
                    shards = [
                        jax.device_put(a[c * per:(c + 1) * per], devices[c])
                        for c in range(N_CORES)
                    ]
                    put.append(jax.make_array_from_single_device_arrays(
                        a.shape, self.sharding, shards))
                jax.block_until_ready(put)
                args = put
            except Exception:
                pass
        return args

    def run(self, args):
        outs = self.fn(*args)
        self.jax.block_until_ready(outs)
        return outs

    def results(self, outs):
        res = []
        for c in range(N_CORES):
            res.append({
                name: np.asarray(outs[i]).reshape(N_CORES, *self.out_avals[i].shape)[c]
                for i, name in enumerate(self.out_names)
            })
        return res


def _get_runner(Ks, reps=1, body_mult=1, variant=0, **bk):
    key = (tuple(Ks), reps, body_mult, variant, tuple(sorted(bk.items())))
    if key not in _RUNNER_CACHE:
        _install_compat()
        _RUNNER_CACHE[key] = _Runner(
            _build_nc(tuple(Ks), reps, body_mult, variant, **bk))
    return _RUNNER_CACHE[key]


# ---------------------------------------------------------------------------
# host-side shard construction (degree-bucketed CSR layout)
#
# Atoms are sorted by in-degree (descending) and dealt round-robin to the 8
# cores, so every core gets an identical degree profile.  Within a core the
# sorted atoms fill a [P=125, APP=500] grid chunk-major: chunk c holds the
# next CAT=25 grid columns of every partition (125*25 = 3125 atoms/core, i.e.
# 25000 atoms globally per chunk).  All atoms of a chunk share a common slot
# width K_c = ceil4(max degree in the chunk) — tight for late (low-degree)
# chunks, so total slots are ~0.55x of a flat max-degree layout.  Padding
# slots carry d=1, qj=0 and contribute exactly 0.
# ---------------------------------------------------------------------------
def _prep(qi, edge_dist, edge_index, q_ref, N, atom_mol_batch):
    qi = np.asarray(qi, np.float32)
    edge_dist = np.asarray(edge_dist, np.float32)
    ii = np.asarray(edge_index[0], np.int64)
    jj = np.asarray(edge_index[1], np.int64)
    # charge-neutrality correction (index-driven segment sum over atoms)
    q_mol = np.bincount(np.asarray(atom_mol_batch, np.int64), weights=qi,
                        minlength=N_MOL).astype(np.float32)
    corr = (q_mol - np.asarray(q_ref, np.float32)) / np.asarray(N, np.float32)
    qi_c = qi - corr[np.asarray(atom_mol_batch, np.int64)]
    qj_c = qi_c[jj]

    counts = np.bincount(ii, minlength=N_ATOMS)
    a_order = np.argsort(-counts, kind="stable")      # degree-descending
    degs = counts[a_order]
    n_chunks = APP // CAT
    cg = N_CORES * P * CAT                            # atoms per global chunk
    Ks = tuple(int(-(-int(degs[c * cg:(c + 1) * cg].max()) // 2) * 2)
               for c in range(n_chunks))
    W = sum(CAT * k for k in Ks)
    offs_c = np.cumsum([0] + [CAT * k for k in Ks])

    # grid coordinates of each atom (by global degree rank).  d and qj are
    # interleaved per chunk in one [rows, 2W] array: chunk c's block is
    # cols [2*offs_c, 2*offs_c + 2*F_c) with d in the first F_c columns.
    rank = np.arange(N_ATOMS, dtype=np.int64)
    core = rank % N_CORES
    r = rank // N_CORES
    c_of = r // (P * CAT)
    w = r % (P * CAT)
    p_of = w % P
    j_of = w // P
    row = core * P + p_of                             # [0, 1000)
    colE = c_of * CAT + j_of                          # E/qic grid column
    kc = np.asarray(Ks, np.int64)[c_of]
    based = row * 2 * W + 2 * offs_c[c_of] + j_of * kc

    # scatter per-atom maps back to atom id
    row_of = np.empty(N_ATOMS, np.int64)
    colE_of = np.empty(N_ATOMS, np.int64)
    based_of = np.empty(N_ATOMS, np.int64)
    fc_of = np.empty(N_ATOMS, np.int64)
    row_of[a_order] = row
    colE_of[a_order] = colE
    based_of[a_order] = based
    fc_of[a_order] = CAT * kc

    # CSR slot assignment per edge (sorted by target atom)
    e_order = np.argsort(ii, kind="stable")
    i_s = ii[e_order]
    csr = np.zeros(N_ATOMS, np.int64)
    np.cumsum(counts[:-1], out=csr[1:])
    slot = np.arange(N_EDGES, dtype=np.int64) - csr[i_s]
    pos_d = based_of[i_s] + slot
    pos_q = pos_d + fc_of[i_s]

    dq = np.zeros((N_CORES * P, 2 * W), np.float32)
    for c in range(n_chunks):
        o = 2 * int(offs_c[c])
        fc = CAT * Ks[c]
        dq[:, o:o + fc] = 1.0                          # d padding (avoid ln(0))
    dq = dq.reshape(-1)
    dq[pos_d] = edge_dist[e_order]
    dq[pos_q] = qj_c[e_order]

    qic_grid = np.zeros((N_CORES * P, APP), np.float32)
    qic_grid[row_of, colE_of] = qi_c
    return {
        "dq": dq.reshape(N_CORES * P, 2 * W),
        "qic": qic_grid,
        "Ks": Ks,
        "row_of": row_of,
        "colE_of": colE_of,
    }


def _shard_maps(prep):
    in_maps = []
    for c in range(N_CORES):
        rs = slice(c * P, (c + 1) * P)
        in_maps.append({
            "dq": prep["dq"][rs],
            "qic": prep["qic"][rs],
        })
    return in_maps


def _unshard(prep, res):
    e_grid = np.concatenate([r["E"].reshape(P, APP) for r in res], axis=0)
    return e_grid[prep["row_of"], prep["colE_of"]].astype(np.float32)


def kernel(qi, edge_dist, edge_index, q_ref, N, atom_mol_batch):
    prep = _prep(qi, edge_dist, edge_index, q_ref, N, atom_mol_batch)
    runner = _get_runner(prep["Ks"])
    args = runner.put_inputs(_shard_maps(prep))
    res = runner.results(runner.run(args))
    return _unshard(prep, res)

